# revision 11
# baseline (speedup 1.0000x reference)
"""Trainium2 Bass kernel for the DParser pairwise-scorer module.

Math (reference):
    Wa, Wb = W1[:768], W1[768:]
    A = hidden @ Wa;  B = hidden @ Wb                       # [1024, 128]
    scores[i, j] = sum_h relu(A[i,h] + B[j,h] + b1[h]) * W2[h] + b2
    out = concat([diag(scores)[None, :], scores with -inf diagonal], 0)

Kernel strategy (8 NeuronCores, row-sharded pair grid):
  * core r owns rows gi in [128r, 128r+128)
  * identity: W2[h]*relu(x) = sign(W2[h]) * relu(|W2[h]|*x), so |W2| is
    folded into Wa/Wb/b1 host-side; on-chip reduction weights are +/-1.
  * inputs are pre-transposed/bf16-cast host-side; on chip, per core:
      Atil_T[h, i] = (hblk @ Wa')^T + b1'        [128h x 128i]  (bf16 PE, f32 out)
      Btil_T[h, j] = (hidden @ Wb')^T            [128h x 1024j] (bf16 PE, bf16 out)
      per i: act[h, j] = relu(Btil_T[h, j] + Atil_T[h, i]), bf16, produced
             whole-row: ScalarE takes 5 of 16 rows, DVE the rest.
             Reduction over h via PE: lhsT = sgn * onehot(i % 32)  [128, 32]
             so row i%32 of the 32-partition output group gets the score row
             and the other 31 rows get zeros; accumulating 32 such matmuls
             (start at i%32==0, stop at i%32==31) fills a [32, 512] PSUM
             block. 4 groups of 32 fill all 128 partitions.
      epilogue: psum + b2 per j-half, DMA out.  The -inf diagonal is filled
      host-side during unshard (positions are statically known constants).
      diag row (row 0) computed separately from the per-core column block.
"""

import numpy as np
import ml_dtypes

import concourse.bass as bass
import concourse.mybir as mybir
import concourse.tile as tile
from concourse import bacc

N = 1024
D = 768
H = 128
NCORES = 8
RB = N // NCORES  # 128 rows per core
KC = D // 128     # 6 contraction chunks
# act rows are produced whole, one engine-op per i: ScalarE takes 3 of 8,
# DVE (2x bf16) takes 5 of 8 -- full-row ops amortize the per-op fixed
# costs (~222 ACT / ~58 DVE init cycles) better than column-splitting.
# (GpSimd measured ~3us per tensor_scalar op on HW and contends with DVE
# for SBUF ports -- do not use it for act-gen)

F32 = mybir.dt.float32
BF16 = mybir.dt.bfloat16
AF = mybir.ActivationFunctionType
OP = mybir.AluOpType


def build_program(b2val: float) -> bass.Bass:
    nc = bacc.Bacc("TRN2")

    hT = nc.declare_dram_parameter("hT", [D, N], BF16, isOutput=False)
    hbT = nc.declare_dram_parameter("hbT", [D, RB], BF16, isOutput=False)
    wab = nc.declare_dram_parameter("wab", [D, 2 * H], BF16, isOutput=False)
    b1s = nc.declare_dram_parameter("b1s", [H, 2], F32, isOutput=False)
    sgnm = nc.declare_dram_parameter("sgnm", [H, 32 * 32], BF16, isOutput=False)
    out_rows = nc.declare_dram_parameter("out_rows", [RB, N], F32, isOutput=True)
    out_diag = nc.declare_dram_parameter("out_diag", [1, RB], F32, isOutput=True)

    with tile.TileContext(nc) as tc:
        with (
            tc.tile_pool(name="const", bufs=1) as cpool,
            tc.tile_pool(name="act", bufs=4) as apool,
            tc.tile_pool(name="psA", bufs=1, space="PSUM") as psA,
            tc.tile_pool(name="psB", bufs=2, space="PSUM") as psB,
            tc.tile_pool(name="psS", bufs=1, space="PSUM") as psS,
            tc.tile_pool(name="psD", bufs=1, space="PSUM") as psD,
            tc.tile_pool(name="outp", bufs=2) as opool,
        ):
            # ---- loads (small operands first; hT is the bulk) ----
            hbT_sb = cpool.tile([128, KC, RB], BF16, tag="hbT_sb")
            wab_sb = cpool.tile([128, KC, 2 * H], BF16, tag="wab_sb")
            b1s_sb = cpool.tile([128, 2], F32, tag="b1s_sb")
            sgn_sb = cpool.tile([128, 1], BF16, tag="sgn_sb")
            sgnm_sb = cpool.tile([128, 32, 32], BF16, tag="sgnm_sb")
            hT_sb = cpool.tile([128, KC, N], BF16, tag="hT_sb")

            # consolidated loads: one dma_start per tensor (issue cost on the
            # sequencer is ~0.6us each), hT split across two engines
            nc.sync.dma_start(out=wab_sb[:], in_=wab.rearrange("(k p) c -> p k c", p=128))
            nc.sync.dma_start(out=hbT_sb[:], in_=hbT.rearrange("(k p) c -> p k c", p=128))
            nc.scalar.dma_start(out=b1s_sb[:], in_=b1s[:, :])
            nc.scalar.dma_start(out=sgnm_sb[:], in_=sgnm.rearrange("h (m c) -> h m c", m=32))
            h3 = KC // 2
            nc.sync.dma_start(
                out=hT_sb[:, 0:h3, :],
                in_=hT[0:h3 * 128, :].rearrange("(k p) c -> p k c", p=128))
            nc.scalar.dma_start(
                out=hT_sb[:, h3:KC, :],
                in_=hT[h3 * 128:D, :].rearrange("(k p) c -> p k c", p=128))
            b1c_sb = b1s_sb[:, 0:1]
            nc.vector.tensor_copy(sgn_sb[:], b1s_sb[:, 1:2])

            # ---- PE warm-up: matmuls on an unread scratch tile, no input
            # deps, so the PE HAM clock-gate opens (1.2->2.4GHz) while the
            # input DMAs are still in flight ----
            warm_sb = cpool.tile([128, 128], BF16, tag="warm_sb")
            nc.vector.memset(warm_sb[:], 0.0)
            warm_ps = psD.tile([128, 128], F32, tag="warm_ps")
            for _ in range(64):
                nc.tensor.matmul(warm_ps[:], warm_sb[:], warm_sb[:],
                                 start=True, stop=True)

            # ---- prologue matmuls (all bf16, fp32 PSUM accumulate) ----
            # Atil_T[h, i] = sum_d Wa'[d, h] * hblkT[d, i]   (+ b1' bias)
            at_ps = psA.tile([128, RB], F32, tag="at_ps")
            for k in range(KC):
                nc.tensor.matmul(
                    at_ps[:], wab_sb[:, k, 0:H], hbT_sb[:, k, :],
                    start=(k == 0), stop=(k == KC - 1),
                )
            at_sb = cpool.tile([128, RB], F32, tag="at_sb")
            nc.scalar.activation(at_sb[:], at_ps[:], AF.Identity, bias=b1c_sb, scale=1.0)

            # Btil_T[h, j] = sum_d Wb'[d, h] * hT[d, j]  -> bf16 SBUF
            # (computed before bd/diag: the whole main loop depends on it)
            bt_sb = cpool.tile([128, N], BF16, tag="bt_sb")
            for jc in range(2):
                bt_ps = psB.tile([128, 512], F32, tag="bt_ps")
                for k in range(KC):
                    nc.tensor.matmul(
                        bt_ps[:], wab_sb[:, k, H:2 * H],
                        hT_sb[:, k, jc * 512:(jc + 1) * 512],
                        start=(k == 0), stop=(k == KC - 1),
                    )
                if jc == 0:
                    nc.scalar.copy(bt_sb[:, 0:512], bt_ps[:])
                else:
                    nc.vector.tensor_copy(bt_sb[:, 512:1024], bt_ps[:])

            # Btil for this core's own column block (for the diag row):
            # bd[h, i] = sum_d Wb'[d, h] * hblkT[d, i]
            bd_ps = psA.tile([128, RB], F32, tag="bd_ps")
            for k in range(KC):
                nc.tensor.matmul(
                    bd_ps[:], wab_sb[:, k, H:2 * H], hbT_sb[:, k, :],
                    start=(k == 0), stop=(k == KC - 1),
                )
            bd_sb = cpool.tile([128, RB], F32, tag="bd_sb")
            nc.vector.tensor_copy(bd_sb[:], bd_ps[:])

            # diag row: actd[h, i] = relu(bd + at); diag[i] = s^T @ actd + b2
            sumd = apool.tile([128, RB], F32, tag="sumd")
            nc.vector.tensor_tensor(out=sumd[:], in0=bd_sb[:], in1=at_sb[:], op=OP.add)
            actd = apool.tile([128, RB], BF16, tag="actd")
            nc.scalar.activation(actd[:], sumd[:], AF.Relu, bias=0.0, scale=1.0)
            diag_ps = psD.tile([1, RB], F32, tag="diag_ps")
            nc.tensor.matmul(diag_ps[:], sgn_sb[:], actd[:], start=True, stop=True)
            diag_sb = opool.tile([1, RB], F32, tag="diag_sb")
            nc.scalar.activation(diag_sb[:], diag_ps[:], AF.Copy, bias=b2val, scale=1.0)
            nc.sync.dma_start(out=out_diag[:, :], in_=diag_sb[:])

            # ---- main loop ----
            score_ps = psS.tile([128, N], F32, tag="score_ps")
            for i in range(RB):
                g, m = divmod(i, 32)
                act_t = apool.tile([128, N], BF16, tag="act_t")
                bias_i = at_sb[:, i:i + 1]
                if i % 16 in (0, 3, 6, 9, 12):
                    # ScalarE (measured ~1.1us/row) takes 5 of every 16 rows
                    nc.scalar.activation(
                        act_t[:], bt_sb[:], AF.Relu, bias=bias_i, scale=1.0,
                    )
                else:
                    # DVE 2x bf16 (measured ~0.49us/row) takes the other 11
                    nc.vector.tensor_scalar(
                        act_t[:], bt_sb[:],
                        scalar1=bias_i, scalar2=0.0, op0=OP.add, op1=OP.max,
                    )
                for jc in range(2):
                    nc.tensor.matmul(
                        score_ps[g * 32:(g + 1) * 32, jc * 512:(jc + 1) * 512],
                        sgnm_sb[:, m, :], act_t[:, jc * 512:(jc + 1) * 512],
                        start=(m == 0), stop=(m == 31),
                        skip_group_check=True,
                        tile_position=(0, g * 32),
                    )

            # ---- epilogue: + b2 and store, halves on ScalarE and DVE in
            # parallel (the -inf diagonal is filled host-side during unshard;
            # positions are statically known constants) ----
            topq = opool.tile([64, N], F32, tag="topq")
            botq = opool.tile([64, N], F32, tag="botq")
            nc.scalar.activation(topq[:], score_ps[0:64, :], AF.Copy,
                                 bias=b2val, scale=1.0)
            nc.vector.tensor_scalar(botq[:], score_ps[64:128, :],
                                    scalar1=b2val, scalar2=None, op0=OP.add)
            nc.sync.dma_start(out=out_rows[0:64, :], in_=topq[:])
            nc.scalar.dma_start(out=out_rows[64:128, :], in_=botq[:])

    nc.finalize()
    return nc


def make_in_maps(hidden, W1, b1, W2, b2):
    hidden = np.ascontiguousarray(np.asarray(hidden, dtype=np.float32))
    W1 = np.asarray(W1, dtype=np.float32)
    b1 = np.asarray(b1, dtype=np.float32)
    W2 = np.asarray(W2, dtype=np.float32)

    absw = np.abs(W2)
    s = np.sign(W2).astype(np.float32)
    # sign(0) = 0 would drop the h lane entirely, which is also correct
    # (w=0 contributes nothing), so no special-casing needed.
    Wap = W1[:D] * absw[None, :]   # [768, 128]
    Wbp = W1[D:] * absw[None, :]   # [768, 128]
    b1p = (b1 * absw).astype(np.float32)

    bf = ml_dtypes.bfloat16
    hT = np.ascontiguousarray(hidden.T.astype(bf))       # [768, 1024] bf16
    wab = np.ascontiguousarray(
        np.concatenate([Wap, Wbp], axis=1).astype(bf))   # [768, 256] bf16
    b1sv = np.ascontiguousarray(
        np.stack([b1p, s], axis=1).astype(np.float32))   # [128, 2] f32

    # sgnm[:, m, c] = s * (c == m): one-hot-masked sign columns.
    sgnm = np.zeros((H, 32, 32), dtype=bf)
    for m in range(32):
        sgnm[:, m, m] = s.astype(bf)
    sgnm = np.ascontiguousarray(sgnm.reshape(H, 32 * 32))

    in_maps = []
    for r in range(NCORES):
        g0 = r * RB
        in_maps.append({
            "hT": hT,
            "hbT": np.ascontiguousarray(hT[:, g0:g0 + RB]),
            "wab": wab,
            "b1s": b1sv,
            "sgnm": sgnm,
        })
    return in_maps


def kernel(hidden, W1, b1, W2, b2):
    from concourse.bass_utils import run_bass_kernel_spmd

    b2val = float(np.asarray(b2, dtype=np.float32)[0])
    nc = build_program(b2val)
    in_maps = make_in_maps(hidden, W1, b1, W2, b2)
    res = run_bass_kernel_spmd(nc, in_maps, core_ids=list(range(NCORES))).results

    out = np.empty((N + 1, N), dtype=np.float32)
    for r in range(NCORES):
        out[0, r * RB:(r + 1) * RB] = res[r]["out_diag"][0]
        out[1 + r * RB:1 + (r + 1) * RB, :] = res[r]["out_rows"]
    out[1:, :][np.arange(N), np.arange(N)] = -np.inf
    return out


# revision 12
# speedup vs baseline: 1.1127x; 1.1127x over previous
"""Trainium2 Bass kernel for the DParser pairwise-scorer module.

Math (reference):
    Wa, Wb = W1[:768], W1[768:]
    A = hidden @ Wa;  B = hidden @ Wb                       # [1024, 128]
    scores[i, j] = sum_h relu(A[i,h] + B[j,h] + b1[h]) * W2[h] + b2
    out = concat([diag(scores)[None, :], scores with -inf diagonal], 0)

Kernel strategy (8 NeuronCores, row-sharded pair grid):
  * core r owns rows gi in [128r, 128r+128)
  * identity: W2[h]*relu(x) = sign(W2[h]) * relu(|W2[h]|*x), so |W2| is
    folded into Wa/Wb/b1 host-side; on-chip reduction weights are +/-1.
  * inputs are pre-transposed/bf16-cast host-side; on chip, per core:
      Atil_T[h, i] = (hblk @ Wa')^T + b1'        [128h x 128i]  (bf16 PE, f32 out)
      Btil_T[h, j] = (hidden @ Wb')^T            [128h x 1024j] (bf16 PE, bf16 out)
      per i: act[h, j] = relu(Btil_T[h, j] + Atil_T[h, i]), bf16, produced
             whole-row: ScalarE takes 5 of 16 rows, DVE the rest.
             Reduction over h via PE: lhsT = sgn * onehot(i % 32)  [128, 32]
             so row i%32 of the 32-partition output group gets the score row
             and the other 31 rows get zeros; accumulating 32 such matmuls
             (start at i%32==0, stop at i%32==31) fills a [32, 512] PSUM
             block. 4 groups of 32 fill all 128 partitions.
      epilogue: psum + b2 per j-half, DMA out.  The -inf diagonal is filled
      host-side during unshard (positions are statically known constants).
      diag row (row 0) computed separately from the per-core column block.
"""

import numpy as np
import ml_dtypes

import concourse.bass as bass
import concourse.mybir as mybir
import concourse.tile as tile
from concourse import bacc

N = 1024
D = 768
H = 128
NCORES = 8
RB = N // NCORES  # 128 rows per core
KC = D // 128     # 6 contraction chunks
# act rows are produced whole, one engine-op per i: ScalarE takes 3 of 8,
# DVE (2x bf16) takes 5 of 8 -- full-row ops amortize the per-op fixed
# costs (~222 ACT / ~58 DVE init cycles) better than column-splitting.
# (GpSimd measured ~3us per tensor_scalar op on HW and contends with DVE
# for SBUF ports -- do not use it for act-gen)

F32 = mybir.dt.float32
BF16 = mybir.dt.bfloat16
AF = mybir.ActivationFunctionType
OP = mybir.AluOpType


def build_program(b2val: float) -> bass.Bass:
    nc = bacc.Bacc("TRN2")

    # inputs are host-shuffled to [128 partitions, chunk, cols] so every DMA
    # descriptor is one partition's full contiguous row (4-12KB descriptors
    # instead of 2KB strided ones -- the head was DMA-descriptor-bound)
    hT = nc.declare_dram_parameter("hT", [128, KC * N], BF16, isOutput=False)
    hbT = nc.declare_dram_parameter("hbT", [128, KC * RB], BF16, isOutput=False)
    wab = nc.declare_dram_parameter("wab", [128, KC * 2 * H], BF16, isOutput=False)
    b1s = nc.declare_dram_parameter("b1s", [H, 2], F32, isOutput=False)
    sgnm = nc.declare_dram_parameter("sgnm", [H, 32 * 32], BF16, isOutput=False)
    out_rows = nc.declare_dram_parameter("out_rows", [RB, N], F32, isOutput=True)
    out_diag = nc.declare_dram_parameter("out_diag", [1, RB], F32, isOutput=True)

    with tile.TileContext(nc) as tc:
        with (
            tc.tile_pool(name="const", bufs=1) as cpool,
            tc.tile_pool(name="act", bufs=8) as apool,
            tc.tile_pool(name="psA", bufs=1, space="PSUM") as psA,
            tc.tile_pool(name="psB", bufs=2, space="PSUM") as psB,
            tc.tile_pool(name="psS", bufs=1, space="PSUM") as psS,
            tc.tile_pool(name="psD", bufs=1, space="PSUM") as psD,
            tc.tile_pool(name="outp", bufs=2) as opool,
        ):
            # ---- loads (small operands first; hT is the bulk) ----
            hbT_sb = cpool.tile([128, KC, RB], BF16, tag="hbT_sb")
            wab_sb = cpool.tile([128, KC, 2 * H], BF16, tag="wab_sb")
            b1s_sb = cpool.tile([128, 2], F32, tag="b1s_sb")
            sgn_sb = cpool.tile([128, 1], BF16, tag="sgn_sb")
            sgnm_sb = cpool.tile([128, 32, 32], BF16, tag="sgnm_sb")
            hT_sb = cpool.tile([128, KC, N], BF16, tag="hT_sb")

            # consolidated loads: one dma_start per tensor (issue cost on the
            # sequencer is ~0.6us each), hT split across two engines
            nc.sync.dma_start(out=wab_sb[:], in_=wab.rearrange("p (k c) -> p k c", k=KC))
            nc.sync.dma_start(out=hbT_sb[:], in_=hbT.rearrange("p (k c) -> p k c", k=KC))
            nc.scalar.dma_start(out=b1s_sb[:], in_=b1s[:, :])
            nc.scalar.dma_start(out=sgnm_sb[:], in_=sgnm.rearrange("h (m c) -> h m c", m=32))
            hT3 = hT.rearrange("p (k c) -> p k c", k=KC)
            for half, eng in ((0, nc.sync), (1, nc.scalar), (2, nc.sync)):
                eng.dma_start(
                    out=hT_sb[:, 2 * half:2 * half + 2, :],
                    in_=hT3[:, 2 * half:2 * half + 2, :])
            b1c_sb = b1s_sb[:, 0:1]
            nc.vector.tensor_copy(sgn_sb[:], b1s_sb[:, 1:2])

            # ---- PE warm-up: matmuls on an unread scratch tile, no input
            # deps, so the PE HAM clock-gate opens (1.2->2.4GHz) while the
            # input DMAs are still in flight ----
            warm_sb = cpool.tile([128, 128], BF16, tag="warm_sb")
            nc.vector.memset(warm_sb[:], 0.0)
            warm_ps = psD.tile([128, 128], F32, tag="warm_ps")
            for _ in range(64):
                nc.tensor.matmul(warm_ps[:], warm_sb[:], warm_sb[:],
                                 start=True, stop=True)

            # ---- prologue matmuls (all bf16, fp32 PSUM accumulate) ----
            # Atil_T[h, i] = sum_d Wa'[d, h] * hblkT[d, i]   (+ b1' bias)
            at_ps = psA.tile([128, RB], F32, tag="at_ps")
            for k in range(KC):
                nc.tensor.matmul(
                    at_ps[:], wab_sb[:, k, 0:H], hbT_sb[:, k, :],
                    start=(k == 0), stop=(k == KC - 1),
                )
            at_sb = cpool.tile([128, RB], F32, tag="at_sb")
            nc.scalar.activation(at_sb[:], at_ps[:], AF.Identity, bias=b1c_sb, scale=1.0)

            # Btil_T[h, j] = sum_d Wb'[d, h] * hT[d, j]  -> bf16 SBUF
            # (computed before bd/diag: the whole main loop depends on it)
            bt_sb = cpool.tile([128, N], BF16, tag="bt_sb")
            for jc in range(2):
                bt_ps = psB.tile([128, 512], F32, tag="bt_ps")
                for k in range(KC):
                    nc.tensor.matmul(
                        bt_ps[:], wab_sb[:, k, H:2 * H],
                        hT_sb[:, k, jc * 512:(jc + 1) * 512],
                        start=(k == 0), stop=(k == KC - 1),
                    )
                if jc == 0:
                    nc.scalar.copy(bt_sb[:, 0:512], bt_ps[:])
                else:
                    nc.vector.tensor_copy(bt_sb[:, 512:1024], bt_ps[:])

            # Btil for this core's own column block (for the diag row):
            # bd[h, i] = sum_d Wb'[d, h] * hblkT[d, i]
            bd_ps = psA.tile([128, RB], F32, tag="bd_ps")
            for k in range(KC):
                nc.tensor.matmul(
                    bd_ps[:], wab_sb[:, k, H:2 * H], hbT_sb[:, k, :],
                    start=(k == 0), stop=(k == KC - 1),
                )
            bd_sb = cpool.tile([128, RB], F32, tag="bd_sb")
            nc.vector.tensor_copy(bd_sb[:], bd_ps[:])

            # diag row: actd[h, i] = relu(bd + at); diag[i] = s^T @ actd + b2
            sumd = apool.tile([128, RB], F32, tag="sumd")
            nc.vector.tensor_tensor(out=sumd[:], in0=bd_sb[:], in1=at_sb[:], op=OP.add)
            actd = apool.tile([128, RB], BF16, tag="actd")
            nc.scalar.activation(actd[:], sumd[:], AF.Relu, bias=0.0, scale=1.0)
            diag_ps = psD.tile([1, RB], F32, tag="diag_ps")
            nc.tensor.matmul(diag_ps[:], sgn_sb[:], actd[:], start=True, stop=True)
            diag_sb = opool.tile([1, RB], F32, tag="diag_sb")
            nc.scalar.activation(diag_sb[:], diag_ps[:], AF.Copy, bias=b2val, scale=1.0)
            nc.sync.dma_start(out=out_diag[:, :], in_=diag_sb[:])

            # ---- main loop ----
            score_ps = psS.tile([128, N], F32, tag="score_ps")
            for i in range(RB):
                g, m = divmod(i, 32)
                act_t = apool.tile([128, N], BF16, tag="act_t")
                bias_i = at_sb[:, i:i + 1]
                if i % 16 in (0, 3, 6, 9, 12):
                    # ScalarE (measured ~1.1us/row) takes 5 of every 16 rows
                    nc.scalar.activation(
                        act_t[:], bt_sb[:], AF.Relu, bias=bias_i, scale=1.0,
                    )
                else:
                    # DVE 2x bf16 (measured ~0.49us/row) takes the other 11
                    nc.vector.tensor_scalar(
                        act_t[:], bt_sb[:],
                        scalar1=bias_i, scalar2=0.0, op0=OP.add, op1=OP.max,
                    )
                for jc in range(2):
                    nc.tensor.matmul(
                        score_ps[g * 32:(g + 1) * 32, jc * 512:(jc + 1) * 512],
                        sgnm_sb[:, m, :], act_t[:, jc * 512:(jc + 1) * 512],
                        start=(m == 0), stop=(m == 31),
                        skip_group_check=True,
                        tile_position=(0, g * 32),
                    )

            # ---- epilogue: + b2 and store, halves on ScalarE and DVE in
            # parallel (the -inf diagonal is filled host-side during unshard;
            # positions are statically known constants) ----
            topq = opool.tile([64, N], F32, tag="topq")
            botq = opool.tile([64, N], F32, tag="botq")
            nc.scalar.activation(topq[:], score_ps[0:64, :], AF.Copy,
                                 bias=b2val, scale=1.0)
            nc.vector.tensor_scalar(botq[:], score_ps[64:128, :],
                                    scalar1=b2val, scalar2=None, op0=OP.add)
            nc.sync.dma_start(out=out_rows[0:64, :], in_=topq[:])
            nc.scalar.dma_start(out=out_rows[64:128, :], in_=botq[:])

    nc.finalize()
    return nc


def make_in_maps(hidden, W1, b1, W2, b2):
    hidden = np.ascontiguousarray(np.asarray(hidden, dtype=np.float32))
    W1 = np.asarray(W1, dtype=np.float32)
    b1 = np.asarray(b1, dtype=np.float32)
    W2 = np.asarray(W2, dtype=np.float32)

    absw = np.abs(W2)
    s = np.sign(W2).astype(np.float32)
    # sign(0) = 0 would drop the h lane entirely, which is also correct
    # (w=0 contributes nothing), so no special-casing needed.
    Wap = W1[:D] * absw[None, :]   # [768, 128]
    Wbp = W1[D:] * absw[None, :]   # [768, 128]
    b1p = (b1 * absw).astype(np.float32)

    bf = ml_dtypes.bfloat16
    hT = np.ascontiguousarray(hidden.T.astype(bf))       # [768, 1024] bf16
    # shuffle [768, C] -> [128, KC*C]: partition-major, per-partition rows
    # contiguous (chunk k cols at [k*C:(k+1)*C])
    def shuf(a):
        c = a.shape[1]
        return np.ascontiguousarray(
            a.reshape(KC, 128, c).swapaxes(0, 1).reshape(128, KC * c))
    wab = shuf(np.concatenate([Wap, Wbp], axis=1).astype(bf))  # [128, 1536]
    b1sv = np.ascontiguousarray(
        np.stack([b1p, s], axis=1).astype(np.float32))   # [128, 2] f32

    # sgnm[:, m, c] = s * (c == m): one-hot-masked sign columns.
    sgnm = np.zeros((H, 32, 32), dtype=bf)
    for m in range(32):
        sgnm[:, m, m] = s.astype(bf)
    sgnm = np.ascontiguousarray(sgnm.reshape(H, 32 * 32))

    in_maps = []
    for r in range(NCORES):
        g0 = r * RB
        in_maps.append({
            "hT": shuf(hT),
            "hbT": shuf(np.ascontiguousarray(hT[:, g0:g0 + RB])),
            "wab": wab,
            "b1s": b1sv,
            "sgnm": sgnm,
        })
    return in_maps


def kernel(hidden, W1, b1, W2, b2):
    from concourse.bass_utils import run_bass_kernel_spmd

    b2val = float(np.asarray(b2, dtype=np.float32)[0])
    nc = build_program(b2val)
    in_maps = make_in_maps(hidden, W1, b1, W2, b2)
    res = run_bass_kernel_spmd(nc, in_maps, core_ids=list(range(NCORES))).results

    out = np.empty((N + 1, N), dtype=np.float32)
    for r in range(NCORES):
        out[0, r * RB:(r + 1) * RB] = res[r]["out_diag"][0]
        out[1 + r * RB:1 + (r + 1) * RB, :] = res[r]["out_rows"]
    out[1:, :][np.arange(N), np.arange(N)] = -np.inf
    return out
